# revision 7
# baseline (speedup 1.0000x reference)
"""Trainium2 Bass kernel for a dense transformer block (causal MHA + FFN, post-LN).

Sharding over 8 NeuronCores:
  - Attention is tensor-parallel over heads: core c computes heads 2c, 2c+1
    for all 4096 tokens (B*T flattened, batch-major).
  - One AllToAll per batch redistributes the per-head attention outputs so
    core c ends up with the full head-concatenated attention output
    (transposed) for its token half-slices: batch-0 tokens [256c, 256c+256)
    and batch-1 tokens [256c, 256c+256).
  - Wo + residual + LN1 + FFN + residual + LN2 are sequence-parallel: each
    core processes its 2x256 token rows and outputs [512, 1024].

Precision/scaling (rel-err gate 2e-2):
  - QKV / Wo / FFN1 matmuls in fp8e4m3 with DoubleRow (2 k-tiles per pass,
    measured 2x vs bf16 at N=512). Weights pre-scaled x16 host-side to clear
    fp8 subnormals; the scales ride in the activations and wash out in the
    LayerNorms (eps scaled to match). FFN2 stays bf16 (its fp8 noise was the
    dominant error term; bf16 W2/hT cuts total error ~1.4x).
  - Attention internals bf16; softmax denominator via a ones-column in P@V;
    exp restricted to the causally-valid range of diagonal tiles.

Pipelining:
  - qkv for batch 1 is interleaved into attn0's emission (fills the PE's
    exp-wait stalls, keeps the HAM clock warm).
  - Wo matmuls for half 0 are interleaved into attn1's last chunk.
  - All back-half weights prefetched during attention, gated behind the
    attention-critical DMAs by a WAR memset so the startup burst stays small
    (less cross-core skew at the AllToAll).
"""

import sys

sys.path.insert(0, "/opt/trn_rl_repo")

import numpy as np
import ml_dtypes

B, T, E, H = 2, 2048, 1024, 16
HS = E // H  # 64
N_CORES = 8
HPC = H // N_CORES  # heads per core = 2
NTOK = B * T  # 4096
TSL = NTOK // N_CORES  # 512 token rows per core
HSL = TSL // B  # 256 rows per (core, batch) half-slice
EPS = 1e-5

BF16 = ml_dtypes.bfloat16
FP8 = ml_dtypes.float8_e4m3
EO_ = E // 128  # 8
FO_ = 4 * E // 128  # 32

SW = 16.0       # fp8 weight pre-scale (Wq/Wk/Wv/Wo/W1)
S_LN1_IN = 256.0   # wo psum scale: (16 att)(16 Wo)
S_LN1_OUT = 32.0   # LN1 output scale (host-scaled g1, be1)
S_FF = 32.0        # ffn2/W2/residual scale = S_LN1_OUT (W2 in bf16, x1)

_cache = {}


def _build(n_cores=N_CORES):
    import concourse.bass as bass
    import concourse.tile as tile
    import concourse.bacc as bacc
    from concourse import mybir

    BF = mybir.dt.bfloat16
    F32 = mybir.dt.float32
    F8 = mybir.dt.float8e4
    AF = mybir.ActivationFunctionType
    OP = mybir.AluOpType
    DR = mybir.MatmulPerfMode.DoubleRow

    nc = bacc.Bacc("TRN2", target_bir_lowering=False, debug=False,
                   num_devices=n_cores)

    EO = E // 128            # 8 chunks of the embedding dim
    FO = 4 * E // 128        # 32 chunks of the FFN hidden dim
    TC = T // 512            # 4 t-chunks of 512 per batch
    M2N = HSL // 128         # 2 row-subtiles per half

    xT_d = nc.dram_tensor("xT", [128, NTOK // 512, EO, 512], F8,
                          kind="ExternalInput")
    xsl_d = nc.dram_tensor("x_slice", [128, TSL // 128, E], F32,
                           kind="ExternalInput")
    wq_d = nc.dram_tensor("wq", [128, EO, HPC * HS], F8, kind="ExternalInput")
    wk_d = nc.dram_tensor("wk", [128, EO, HPC * HS], F8, kind="ExternalInput")
    wv_d = nc.dram_tensor("wv", [128, EO, HPC * HS], F8, kind="ExternalInput")
    wo_d = nc.dram_tensor("wo", [128, EO, E], F8, kind="ExternalInput")
    w1_d = nc.dram_tensor("w1", [128, FO, EO, 128], F8, kind="ExternalInput")
    w2_d = nc.dram_tensor("w2", [128, FO, E], BF, kind="ExternalInput")
    b1s_d = nc.dram_tensor("b1s", [128, FO], F32, kind="ExternalInput")
    bo_d = nc.dram_tensor("bo", [E], BF, kind="ExternalInput")      # 256*bo
    b2r_d = nc.dram_tensor("b2r", [1, E], BF, kind="ExternalInput")  # 32*b2
    g1_d = nc.dram_tensor("g1", [E], BF, kind="ExternalInput")      # 32*g1
    be1_d = nc.dram_tensor("be1", [E], BF, kind="ExternalInput")    # 32*be1
    g2_d = nc.dram_tensor("g2", [E], BF, kind="ExternalInput")
    be2_d = nc.dram_tensor("be2", [E], BF, kind="ExternalInput")
    masks_d = nc.dram_tensor("masks", [128, 128], BF, kind="ExternalInput")
    idb_d = nc.dram_tensor("id_bf", [128, 128], BF, kind="ExternalInput")
    idf_d = nc.dram_tensor("id_f32", [128, 128], F32, kind="ExternalInput")
    out_d = nc.dram_tensor("out", [TSL, E], BF, kind="ExternalOutput")

    def bcast_ap(d, n):
        a = d.ap()
        return bass.AP(tensor=a.tensor, offset=a.offset, ap=[[0, 128], [1, n]])

    with tile.TileContext(nc) as tc:
        with tc.tile_pool(name="dram", bufs=1, space="DRAM") as dram, \
             tc.tile_pool(name="consts", bufs=1) as consts:

            a2a_in = [dram.tile([n_cores, 128, HSL], BF, name=f"a2a_in{b}")
                      for b in range(B)]
            a2a_out = [dram.tile([n_cores, 128, HSL], BF, name=f"a2a_out{b}")
                       for b in range(B)]

            # ---- attention-critical DMAs first on the sync queue --------
            wq_sb = consts.tile([128, EO, HPC * HS], F8)
            nc.sync.dma_start(wq_sb[:], wq_d.ap())
            wk_sb = consts.tile([128, EO, HPC * HS], F8)
            wv_sb = consts.tile([128, EO, HPC * HS], F8)
            masks_sb = consts.tile([128, 128], BF)
            nc.scalar.dma_start(masks_sb[:], masks_d.ap())
            idb_sb = consts.tile([128, 128], BF)
            nc.scalar.dma_start(idb_sb[:], idb_d.ap())
            idf_sb = consts.tile([128, 128], F32)
            nc.gpsimd.dma_start(idf_sb[:], idf_d.ap())
            eps1_sb = consts.tile([128, 1], F32)
            nc.vector.memset(eps1_sb[:], S_LN1_IN * S_LN1_IN * EPS)
            eps2_sb = consts.tile([128, 1], F32)
            nc.vector.memset(eps2_sb[:], S_FF * S_FF * EPS)
            ones_row = consts.tile([1, 128], BF)
            nc.vector.memset(ones_row[:], 1.0)
            # small broadcast vectors on the gpsimd queue (tiny reads)
            b1_sb = consts.tile([128, FO], F32)
            nc.gpsimd.dma_start(b1_sb[:], b1s_d.ap())
            bo_bc = consts.tile([128, E], BF)
            nc.gpsimd.dma_start(bo_bc[:], bcast_ap(bo_d, E))
            b2r_sb = consts.tile([1, E], BF)
            nc.gpsimd.dma_start(b2r_sb[:], b2r_d.ap())
            g1_bc = consts.tile([128, E], BF)
            nc.gpsimd.dma_start(g1_bc[:], bcast_ap(g1_d, E))
            be1_bc = consts.tile([128, E], BF)
            nc.gpsimd.dma_start(be1_bc[:], bcast_ap(be1_d, E))
            g2_bc = consts.tile([128, E], BF)
            nc.gpsimd.dma_start(g2_bc[:], bcast_ap(g2_d, E))
            be2_bc = consts.tile([128, E], BF)
            nc.gpsimd.dma_start(be2_bc[:], bcast_ap(be2_d, E))

            # back-half weights; DMAs issued after qkv0 via WAR memset gate
            # (w1 is streamed inside ffn1 -- read exactly once)
            wo_sb = consts.tile([128, EO, E], F8)
            w2_sb = consts.tile([128, FO, E], BF)
            xpb_sb = consts.tile([128, TSL // 128, E], F32)  # 256*(x+bo)

            # x1f lives across attention (wo0 is interleaved into attn1)
            bh1_cm = tc.tile_pool(name="bh1", bufs=1)
            bh1 = bh1_cm.__enter__()
            x1f = bh1.tile([128, TSL // 128, E], F32, tag="x1f")
            hcT8 = [bh1.tile([128, EO, HSL], F8, tag=f"hcT8_{h2}",
                             name=f"hcT8_{h2}") for h2 in range(B)]

            with tc.tile_pool(name="att_big", bufs=1) as att_big, \
                 tc.tile_pool(name="att_qkv", bufs=2) as att_qkv, \
                 tc.tile_pool(name="att_pt", bufs=3) as att_pt, \
                 tc.tile_pool(name="att_small", bufs=4) as att_small, \
                 tc.tile_pool(name="ps_big", bufs=2, space="PSUM") as ps_big, \
                 tc.tile_pool(name="ps_small", bufs=2, space="PSUM") as ps_small, \
                 tc.tile_pool(name="ps_av", bufs=2, space="PSUM") as ps_av:
                ps_qk = ps_s = ps_big          # 2x 2-bank slots (tag "qs")
                ps_v = ps_tp = ps_wo0 = ps_small  # 2x 1-bank 2KB slots

                xT_sb = att_big.tile([128, NTOK // 512, EO, 512], F8, tag="xT")
                nc.sync.dma_start(xT_sb[:, 0], xT_d.ap()[:, 0])
                nc.sync.dma_start(wk_sb[:], wk_d.ap())
                nc.sync.dma_start(wv_sb[:], wv_d.ap())
                for sl_i in range(1, NTOK // 512):
                    nc.sync.dma_start(xT_sb[:, sl_i], xT_d.ap()[:, sl_i])

                qkv_tiles = {}

                def alloc_qkv(b):
                    qkv_tiles[b] = (
                        att_qkv.tile([128, T], BF, tag="qT", name=f"qT{b}"),
                        att_qkv.tile([128, T], BF, tag="kT", name=f"kT{b}"),
                        att_qkv.tile([128, T // 128, 2 * (HS + 1)], BF,
                                     tag="v", name=f"v{b}"),
                    )

                def emit_qk_chunk(b, ci):
                    qT_sb, kT_sb, _ = qkv_tiles[b]
                    cg = b * TC + ci
                    qk_ps = ps_qk.tile([128, 2, 512], F32, tag="qs",
                                       name=f"qk{b}_{ci}")
                    for ep in range(EO // 2):
                        nc.tensor.matmul(
                            qk_ps[:, 0, :], wq_sb[:, 2 * ep:2 * ep + 2, :],
                            xT_sb[:, cg, 2 * ep:2 * ep + 2, :],
                            start=ep == 0, stop=ep == EO // 2 - 1,
                            perf_mode=DR)
                    for ep in range(EO // 2):
                        nc.tensor.matmul(
                            qk_ps[:, 1, :], wk_sb[:, 2 * ep:2 * ep + 2, :],
                            xT_sb[:, cg, 2 * ep:2 * ep + 2, :],
                            start=ep == 0, stop=ep == EO // 2 - 1,
                            perf_mode=DR)
                    nc.vector.tensor_copy(
                        qT_sb[:, 512 * ci:512 * ci + 512], qk_ps[:, 0, :])
                    nc.vector.tensor_copy(
                        kT_sb[:, 512 * ci:512 * ci + 512], qk_ps[:, 1, :])

                def emit_v_unit(b, ci, k2):
                    _, _, v_sb = qkv_tiles[b]
                    cg = b * TC + ci
                    vp = ps_v.tile([128, 512], F32, tag="vtp",
                                   name=f"vp{b}_{ci}_{k2}")
                    vps = vp[:, 0:128]
                    for ep in range(EO // 2):
                        nc.tensor.matmul(
                            vps, xT_sb[:, cg, 2 * ep:2 * ep + 2,
                                       128 * k2:128 * (k2 + 1)],
                            wv_sb[:, 2 * ep:2 * ep + 2, :],
                            start=ep == 0, stop=ep == EO // 2 - 1,
                            perf_mode=DR)
                    ts_ = 4 * ci + k2
                    vrow = v_sb[:, ts_, :]
                    ones_view = bass.AP(
                        tensor=vrow.tensor, offset=vrow.offset + HS,
                        ap=[vrow.ap[0], [HS + 1, 2]])
                    nc.vector.memset(ones_view, 1.0)
                    dst = bass.AP(
                        tensor=vrow.tensor, offset=vrow.offset,
                        ap=[vrow.ap[0], [HS + 1, 2], [1, HS]])
                    nc.vector.tensor_copy(
                        dst, vps.rearrange("p (h d) -> p h d", h=2))

                def emit_wo_group(h2, m2, n):
                    m = M2N * h2 + m2
                    wo_ps = ps_wo0.tile([128, 512], F32, tag="vtp",
                                        name=f"wo{h2}_{m2}_{n}")
                    for hp in range(EO // 2):
                        nc.tensor.matmul(
                            wo_ps[:],
                            hcT8[h2][:, 2 * hp:2 * hp + 2,
                                     128 * m2:128 * (m2 + 1)],
                            wo_sb[:, 2 * hp:2 * hp + 2, 512 * n:512 * (n + 1)],
                            start=hp == 0, stop=hp == EO // 2 - 1,
                            perf_mode=DR)
                    sl = slice(512 * n, 512 * (n + 1))
                    nc.vector.tensor_tensor(
                        x1f[:, m, sl], wo_ps[:], xpb_sb[:, m, sl], OP.add)

                def emit_attn_chunk(b, i, fillers):
                    qT_sb, kT_sb, v_sb = qkv_tiles[b]
                    av_ps = [ps_av.tile([128, 2, 2, HS + 1], F32, tag="av",
                                        name=f"av{b}_{i}_{p}")
                             for p in range(2)]
                    nj = 4 * i + 4
                    nf = len(fillers)
                    fdone = 0
                    for j in range(nj):
                        q = j - 4 * i
                        t0_ = 128 * q if q > 0 else 0
                        s_ps = ps_s.tile([128, 2, 512], F32, tag="qs",
                                         name=f"s{b}_{i}_{j}")
                        for h in range(2):
                            nc.tensor.matmul(
                                s_ps[:, h, :],
                                kT_sb[64 * h:64 * h + 64,
                                      128 * j:128 * j + 128],
                                qT_sb[64 * h:64 * h + 64,
                                      512 * i:512 * i + 512],
                                start=True, stop=True)
                        pt = att_pt.tile([128, 2, 512], BF, tag="pt",
                                         name=f"pt{b}_{i}_{j}")
                        nc.scalar.activation(
                            pt[:, :, t0_:], s_ps[:, :, t0_:], AF.Exp,
                            scale=1.0 / (np.sqrt(HS) * SW * SW))
                        if q >= 0:
                            nc.vector.tensor_tensor(
                                pt[:, :, 128 * q:128 * (q + 1)],
                                pt[:, :, 128 * q:128 * (q + 1)],
                                masks_sb[:, None, :].to_broadcast((128, 2, 128)),
                                OP.mult)
                        for k2 in range(4):
                            if j > 4 * i + k2:
                                continue
                            for h in range(2):
                                nc.tensor.matmul(
                                    av_ps[k2 // 2][:, k2 % 2, h, :],
                                    pt[:, h, 128 * k2:128 * (k2 + 1)],
                                    v_sb[:, j, (HS + 1) * h:(HS + 1) * (h + 1)],
                                    start=(j == 0 and h == 0 and k2 % 2 == 0),
                                    stop=j == 4 * i + k2)
                        # sprinkle independent PE work between j iterations
                        want = nf * (j + 1) // nj
                        while fdone < want:
                            fillers[fdone]()
                            fdone += 1
                    for k2 in range(4):
                        avp = av_ps[k2 // 2][:, k2 % 2, :, :]
                        recip = att_small.tile([128, 2], F32, tag="recip")
                        nc.vector.reciprocal(recip[:], avp[:, :, HS])
                        onorm = att_small.tile([128, 128], BF, tag="onorm")
                        for h in range(2):
                            nc.vector.tensor_scalar_mul(
                                onorm[:, 64 * h:64 * h + 64],
                                avp[:, h, 0:HS], recip[:, h:h + 1])
                        tp = ps_tp.tile([128, 512], F32, tag="vtp",
                                        name=f"tp{b}_{i}_{k2}")
                        tpb = tp[:, 0:128].bitcast(BF)[:, 0:128]
                        nc.tensor.transpose(tpb, onorm[:], idb_sb[:])
                        ot = att_small.tile([128, 128], BF, tag="ot")
                        nc.vector.tensor_copy(ot[:], tpb)
                        g2_ = 512 * i + 128 * k2
                        nc.scalar.dma_start(
                            a2a_in[b][g2_ // HSL, :,
                                      (g2_ % HSL):(g2_ % HSL) + 128],
                            ot[:])

                # ---- batch 0 qkv ----
                with nc.named_scope("qkv0"):
                    alloc_qkv(0)
                    for ci in range(TC):
                        emit_qk_chunk(0, ci)
                        for k2 in range(4):
                            emit_v_unit(0, ci, k2)

                # gate the big prefetches behind qkv0 (vector engine reaches
                # these memsets ~25us in; keeps the startup DMA burst small)
                nc.vector.memset(wo_sb[:, 0, 0:1], 0.0)
                nc.vector.memset(w2_sb[:, 0, 0:1], 0.0)
                nc.vector.memset(xpb_sb[:, 0, 0:1], 0.0)
                nc.sync.dma_start(wo_sb[:], wo_d.ap())
                nc.sync.dma_start(xpb_sb[:], xsl_d.ap())
                nc.sync.dma_start(w2_sb[:], w2_d.ap())

                # ---- attn0 with qkv1 interleaved ----
                alloc_qkv(1)
                with nc.named_scope("attn0"):
                    for i in range(TC):
                        emit_qk_chunk(1, i)
                        fillers = [
                            (lambda ci=i, k2=k2: emit_v_unit(1, ci, k2))
                            for k2 in range(4)]
                        emit_attn_chunk(0, i, fillers)

                with nc.named_scope("a2a0"):
                    nc.gpsimd.collective_compute(
                        "AllToAll", mybir.AluOpType.bypass,
                        replica_groups=[list(range(n_cores))],
                        ins=[a2a_in[0].opt()], outs=[a2a_out[0].opt()])
                hcT0 = att_big.tile([128, EO, HSL], BF, tag="hcT")
                nc.sync.dma_start(
                    hcT0[:], a2a_out[0][:].rearrange("i p t -> p i t"))
                nc.gpsimd.tensor_copy(hcT8[0][:], hcT0[:])
                # xpb = 256*x + 256*bo (DVE, in the attn1 shadow)
                for m_ in range(TSL // 128):
                    nc.vector.tensor_scalar_mul(xpb_sb[:, m_, :],
                                                xpb_sb[:, m_, :], S_LN1_IN)
                    nc.vector.tensor_tensor(xpb_sb[:, m_, :], xpb_sb[:, m_, :],
                                            bo_bc[:], OP.add)

                # ---- attn1 with wo(half 0) interleaved into the last chunk
                with nc.named_scope("attn1"):
                    for i in range(TC):
                        fillers = []
                        if i == TC - 1:
                            fillers = [
                                (lambda m2=m2, n=n: emit_wo_group(0, m2, n))
                                for m2 in range(M2N) for n in range(2)]
                        emit_attn_chunk(1, i, fillers)

                with nc.named_scope("a2a1"):
                    nc.gpsimd.collective_compute(
                        "AllToAll", mybir.AluOpType.bypass,
                        replica_groups=[list(range(n_cores))],
                        ins=[a2a_in[1].opt()], outs=[a2a_out[1].opt()])
                hcT1 = att_big.tile([128, EO, HSL], BF, tag="hcT")
                nc.sync.dma_start(
                    hcT1[:], a2a_out[1][:].rearrange("i p t -> p i t"))
                nc.gpsimd.tensor_copy(hcT8[1][:], hcT1[:])

            # ================= back half =================
            with tc.tile_pool(name="bh2", bufs=1) as bh2, \
                 tc.tile_pool(name="bh_small", bufs=4) as bh_small:

                x1T = bh2.tile([128, EO, TSL], F8, tag="x1T")
                hT = bh2.tile([128, FO, TSL], BF, tag="hT")
                out_bf = bh2.tile([128, TSL // 128, E], BF, tag="out_bf")
                out_dst = out_d.ap().rearrange("(m p) e -> p m e", p=128)

                def layernorm(buf_m, g_bc, be_bc, eps_sb, out_m=None):
                    stats = bh_small.tile([128, 2, 6], F32, tag="stats")
                    for s2 in range(2):
                        nc.vector.bn_stats(stats[:, s2, :],
                                           buf_m[:, 512 * s2:512 * (s2 + 1)])
                    mv = bh_small.tile([128, 2], F32, tag="mv")
                    nc.vector.bn_aggr(mv[:], stats[:])
                    std = bh_small.tile([128, 1], F32, tag="std")
                    nc.scalar.activation(std[:], mv[:, 1:2], AF.Sqrt,
                                         bias=eps_sb[:, 0:1])
                    rstd = bh_small.tile([128, 1], F32, tag="rstd")
                    nc.vector.reciprocal(rstd[:], std[:])
                    nc.vector.tensor_scalar(
                        buf_m[:], buf_m[:], mv[:, 0:1], rstd[:],
                        op0=OP.subtract, op1=OP.mult)
                    nc.vector.tensor_tensor(buf_m[:], buf_m[:], g_bc[:], OP.mult)
                    nc.vector.tensor_tensor(out_m if out_m is not None
                                            else buf_m[:],
                                            buf_m[:], be_bc[:], OP.add)

                with tc.tile_pool(name="ps_wo1", bufs=2, space="PSUM") as ps_wo1, \
                     tc.tile_pool(name="ps_tp2", bufs=2, space="PSUM") as ps_tp2:
                    # LN1 for half 0 (wo adds already done inside attn1)
                    with nc.named_scope("ln1_0"):
                        for m2 in range(M2N):
                            layernorm(x1f[:, m2, :], g1_bc, be1_bc, eps1_sb)
                        for m2 in range(M2N):
                            for eo in range(EO):
                                tp2 = ps_tp2.tile([128, 128], F32, tag="tp2")
                                nc.tensor.transpose(
                                    tp2[:], x1f[:, m2, 128 * eo:128 * (eo + 1)],
                                    idf_sb[:])
                                nc.vector.tensor_copy(
                                    x1T[:, eo, 128 * m2:128 * (m2 + 1)], tp2[:])

                    with nc.named_scope("wo_ln1_1"):
                        for m2 in range(M2N):
                            for n in range(2):
                                m = M2N + m2
                                wo_ps = ps_wo1.tile([128, 512], F32, tag="wo")
                                for hp in range(EO // 2):
                                    nc.tensor.matmul(
                                        wo_ps[:],
                                        hcT8[1][:, 2 * hp:2 * hp + 2,
                                                128 * m2:128 * (m2 + 1)],
                                        wo_sb[:, 2 * hp:2 * hp + 2,
                                              512 * n:512 * (n + 1)],
                                        start=hp == 0, stop=hp == EO // 2 - 1,
                                        perf_mode=DR)
                                sl = slice(512 * n, 512 * (n + 1))
                                nc.vector.tensor_tensor(
                                    x1f[:, m, sl], wo_ps[:], xpb_sb[:, m, sl],
                                    OP.add)
                        for m2 in range(M2N):
                            m = M2N + m2
                            layernorm(x1f[:, m, :], g1_bc, be1_bc, eps1_sb)
                        for m2 in range(M2N):
                            m = M2N + m2
                            for eo in range(EO):
                                tp2 = ps_tp2.tile([128, 128], F32, tag="tp2")
                                nc.tensor.transpose(
                                    tp2[:], x1f[:, m, 128 * eo:128 * (eo + 1)],
                                    idf_sb[:])
                                nc.vector.tensor_copy(
                                    x1T[:, eo, 128 * m:128 * (m + 1)], tp2[:])

                with nc.named_scope("ffn1"), \
                     tc.tile_pool(name="bh_w", bufs=4) as bh_w, \
                     tc.tile_pool(name="ps_f1", bufs=3, space="PSUM") as ps_f1:
                    for fo in range(FO):
                        w1t = bh_w.tile([128, EO, 128], F8, tag="w1t")
                        nc.scalar.dma_start(w1t[:], w1_d.ap()[:, fo])
                        f1_ps = ps_f1.tile([128, TSL], F32, tag="f1")
                        for ep in range(EO // 2):
                            nc.tensor.matmul(
                                f1_ps[:], w1t[:, 2 * ep:2 * ep + 2, :],
                                x1T[:, 2 * ep:2 * ep + 2, :],
                                start=ep == 0, stop=ep == EO // 2 - 1,
                                perf_mode=DR)
                        nc.scalar.activation(hT[:, fo, :], f1_ps[:], AF.Relu,
                                             bias=b1_sb[:, fo:fo + 1],
                                             scale=1.0 / (S_LN1_OUT * SW))

                with nc.named_scope("ffn2_ln2"), \
                     tc.tile_pool(name="ps_f2", bufs=2, space="PSUM") as ps_f2:
                    for m in range(TSL // 128):
                        f2_ps = [ps_f2.tile([128, 512], F32, tag=f"f2_{n}",
                                            name=f"f2_{m}_{n}")
                                 for n in range(2)]
                        for n in range(2):
                            # K=1 matmul seeds the bank with 32*b2 broadcast
                            nc.tensor.matmul(
                                f2_ps[n][:], ones_row[:],
                                b2r_sb[:, 512 * n:512 * (n + 1)],
                                start=True, stop=False)
                        for fo in range(FO):
                            for n in range(2):
                                nc.tensor.matmul(
                                    f2_ps[n][:],
                                    hT[:, fo, 128 * m:128 * (m + 1)],
                                    w2_sb[:, fo, 512 * n:512 * (n + 1)],
                                    start=False, stop=fo == FO - 1)
                        for n in range(2):
                            sl = slice(512 * n, 512 * (n + 1))
                            nc.vector.tensor_tensor(
                                x1f[:, m, sl], f2_ps[n][:], x1f[:, m, sl],
                                OP.add)
                        layernorm(x1f[:, m, :], g2_bc, be2_bc, eps2_sb,
                                  out_m=out_bf[:, m, :])
                        nc.sync.dma_start(out_dst[:, m, :], out_bf[:, m, :])

            bh1_cm.__exit__(None, None, None)

    nc.compile()
    return nc


def _make_in_maps(inputs):
    x = np.asarray(inputs["x"], dtype=np.float32)
    Wq = np.asarray(inputs["Wq"], dtype=np.float32)
    Wk = np.asarray(inputs["Wk"], dtype=np.float32)
    Wv = np.asarray(inputs["Wv"], dtype=np.float32)
    Wo = np.asarray(inputs["Wo"], dtype=np.float32)

    xflat = x.reshape(NTOK, E)
    xT = np.ascontiguousarray(
        xflat.reshape(NTOK // 512, 512, EO_, 128).transpose(3, 0, 2, 1)
    ).astype(FP8)
    wo = np.ascontiguousarray(
        (SW * Wo).reshape(EO_, 128, E).transpose(1, 0, 2)).astype(FP8)
    w1 = np.ascontiguousarray(
        (SW * np.asarray(inputs["W1"], dtype=np.float32))
        .reshape(EO_, 128, FO_, 128).transpose(1, 2, 0, 3)
    ).astype(FP8)
    w2 = np.ascontiguousarray(
        (S_FF * np.asarray(inputs["W2"], dtype=np.float32))
        .reshape(FO_, 128, E).transpose(1, 0, 2)).astype(BF16)
    b1s = np.ascontiguousarray(
        np.asarray(inputs["b1"], dtype=np.float32).reshape(FO_, 128).T)

    srow = np.arange(128)[:, None]
    tcol = np.arange(128)[None, :]
    masks = np.ascontiguousarray((srow <= tcol).astype(np.float32)).astype(BF16)

    ident = np.eye(128, dtype=np.float32)

    common = {
        "xT": xT,
        "wo": wo,
        "w1": w1,
        "w2": w2,
        "b1s": b1s,
        "bo": (S_LN1_IN * np.asarray(inputs["bo"], dtype=np.float32)
               ).astype(BF16),
        "b2r": (S_FF * np.asarray(inputs["b2"], dtype=np.float32)
        ).astype(BF16).reshape(1, E),
        "g1": (S_LN1_OUT * np.asarray(inputs["g1"], dtype=np.float32)
               ).astype(BF16),
        "be1": (S_LN1_OUT * np.asarray(inputs["be1"], dtype=np.float32)
                ).astype(BF16),
        "g2": np.asarray(inputs["g2"], dtype=np.float32).astype(BF16),
        "be2": np.asarray(inputs["be2"], dtype=np.float32).astype(BF16),
        "masks": masks,
        "id_bf": ident.astype(BF16),
        "id_f32": ident,
    }
    in_maps = []
    for c in range(N_CORES):
        m = dict(common)
        def tile_w(W):
            wc = np.concatenate([W[2 * c], W[2 * c + 1]], axis=1)
            return np.ascontiguousarray(
                (SW * wc).reshape(EO_, 128, 128).transpose(1, 0, 2)).astype(FP8)
        m["wq"] = tile_w(Wq)
        m["wk"] = tile_w(Wk)
        m["wv"] = tile_w(Wv)
        rows = np.concatenate([
            xflat[HSL * c:HSL * (c + 1)],
            xflat[T + HSL * c:T + HSL * (c + 1)]], axis=0)
        m["x_slice"] = np.ascontiguousarray(
            rows.reshape(TSL // 128, 128, E).transpose(1, 0, 2))
        in_maps.append(m)
    return in_maps


def _enable_trace_hook():
    """Register the axon NTFF profile hook (synthesize antenv.axon_hooks)."""
    import types
    import antenv  # noqa: F401

    if "antenv.axon_hooks" not in sys.modules:
        mod = types.ModuleType("antenv.axon_hooks")
        mod._hook = None
        mod.set_axon_ntff_profile_hook = lambda h: setattr(mod, "_hook", h)
        mod.get_axon_ntff_profile_hook = lambda: mod._hook
        sys.modules["antenv.axon_hooks"] = mod
        antenv.axon_hooks = mod
    mod = sys.modules["antenv.axon_hooks"]
    if mod.get_axon_ntff_profile_hook() is None:
        if "/root/.axon_site" not in sys.path:
            sys.path.insert(0, "/root/.axon_site")
        from trn_agent_boot.trn_boot import _ntff_profile_via_ctypes
        mod.set_axon_ntff_profile_hook(
            _ntff_profile_via_ctypes("/opt/axon/libaxon_pjrt.so"))


def run(inputs, trace=False):
    """Returns (full_output [B,T,E] f32, BassKernelResults)."""
    from concourse import bass_utils

    if "nc" not in _cache:
        _cache["nc"] = _build()
    nc = _cache["nc"]
    in_maps = _make_in_maps(inputs)
    if trace:
        _enable_trace_hook()
    res = bass_utils.run_bass_kernel_spmd(
        nc, in_maps, core_ids=list(range(N_CORES)), trace=trace)
    full = np.empty((NTOK, E), dtype=np.float32)
    for c in range(N_CORES):
        o = res.results[c]["out"]
        full[HSL * c:HSL * (c + 1)] = o[:HSL]
        full[T + HSL * c:T + HSL * (c + 1)] = o[HSL:]
    return full.reshape(B, T, E), res


def kernel(**inputs):
    out, _ = run(inputs, trace=False)
    return out


# revision 10
# speedup vs baseline: 1.0706x; 1.0706x over previous
"""Trainium2 Bass kernel for a dense transformer block (causal MHA + FFN, post-LN).

Sharding over 8 NeuronCores:
  - Attention is tensor-parallel over heads: core c computes heads 2c, 2c+1
    for all 4096 tokens (B*T flattened, batch-major).
  - One AllToAll per batch redistributes the per-head attention outputs so
    core c ends up with the full head-concatenated attention output
    (transposed) for its token half-slices: batch-0 tokens [256c, 256c+256)
    and batch-1 tokens [256c, 256c+256).
  - Wo + residual + LN1 + FFN + residual + LN2 are sequence-parallel: each
    core processes its 2x256 token rows and outputs [512, 1024].

Precision/scaling (rel-err gate 2e-2):
  - QKV / Wo / FFN1 matmuls in fp8e4m3 with DoubleRow (2 k-tiles per pass,
    measured 2x vs bf16 at N=512). Weights pre-scaled x16 host-side to clear
    fp8 subnormals; the scales ride in the activations and wash out in the
    LayerNorms (eps scaled to match). FFN2 stays bf16 (its fp8 noise was the
    dominant error term; bf16 W2/hT cuts total error ~1.4x).
  - Attention internals bf16; softmax denominator via a ones-column in P@V;
    exp restricted to the causally-valid range of diagonal tiles.

Pipelining:
  - qkv for batch 1 is interleaved into attn0's emission (fills the PE's
    exp-wait stalls, keeps the HAM clock warm).
  - Wo matmuls for half 0 are interleaved into attn1's last chunk.
  - All back-half weights prefetched during attention, gated behind the
    attention-critical DMAs by a WAR memset so the startup burst stays small
    (less cross-core skew at the AllToAll).
"""

import sys

sys.path.insert(0, "/opt/trn_rl_repo")

import numpy as np
import ml_dtypes

B, T, E, H = 2, 2048, 1024, 16
HS = E // H  # 64
N_CORES = 8
HPC = H // N_CORES  # heads per core = 2
NTOK = B * T  # 4096
TSL = NTOK // N_CORES  # 512 token rows per core
HSL = TSL // B  # 256 rows per (core, batch) half-slice
EPS = 1e-5

BF16 = ml_dtypes.bfloat16
FP8 = ml_dtypes.float8_e4m3
EO_ = E // 128  # 8
FO_ = 4 * E // 128  # 32

SW = 16.0       # fp8 weight pre-scale (Wq/Wk/Wv/Wo/W1)
S_LN1_IN = 256.0   # wo psum scale: (16 att)(16 Wo)
S_LN1_OUT = 32.0   # LN1 output scale (host-scaled g1, be1)
S_FF = 32.0        # ffn2/W2/residual scale = S_LN1_OUT (W2 in bf16, x1)

_cache = {}


def _build(n_cores=N_CORES):
    import concourse.bass as bass
    import concourse.tile as tile
    import concourse.bacc as bacc
    from concourse import mybir

    BF = mybir.dt.bfloat16
    F32 = mybir.dt.float32
    F8 = mybir.dt.float8e4
    AF = mybir.ActivationFunctionType
    OP = mybir.AluOpType
    DR = mybir.MatmulPerfMode.DoubleRow

    nc = bacc.Bacc("TRN2", target_bir_lowering=False, debug=False,
                   num_devices=n_cores)

    EO = E // 128            # 8 chunks of the embedding dim
    FO = 4 * E // 128        # 32 chunks of the FFN hidden dim
    TC = T // 512            # 4 t-chunks of 512 per batch
    M2N = HSL // 128         # 2 row-subtiles per half

    xT_d = nc.dram_tensor("xT", [128, NTOK // 512, EO, 512], F8,
                          kind="ExternalInput")
    xsl_d = nc.dram_tensor("x_slice", [128, TSL // 128, E], F32,
                           kind="ExternalInput")
    wq_d = nc.dram_tensor("wq", [128, EO, HPC * HS], F8, kind="ExternalInput")
    wk_d = nc.dram_tensor("wk", [128, EO, HPC * HS], F8, kind="ExternalInput")
    wv_d = nc.dram_tensor("wv", [128, EO, HPC * HS], F8, kind="ExternalInput")
    wo_d = nc.dram_tensor("wo", [128, EO, E], F8, kind="ExternalInput")
    w1_d = nc.dram_tensor("w1", [128, FO, EO, 128], F8, kind="ExternalInput")
    w2_d = nc.dram_tensor("w2", [128, FO, E], BF, kind="ExternalInput")
    b1s_d = nc.dram_tensor("b1s", [128, FO], F32, kind="ExternalInput")
    bo_d = nc.dram_tensor("bo", [E], BF, kind="ExternalInput")      # 256*bo
    b2r_d = nc.dram_tensor("b2r", [1, E], BF, kind="ExternalInput")  # 32*b2
    g1_d = nc.dram_tensor("g1", [E], BF, kind="ExternalInput")      # 32*g1
    be1_d = nc.dram_tensor("be1", [E], BF, kind="ExternalInput")    # 32*be1
    g2_d = nc.dram_tensor("g2", [E], BF, kind="ExternalInput")
    be2_d = nc.dram_tensor("be2", [E], BF, kind="ExternalInput")
    masks_d = nc.dram_tensor("masks", [128, 128], BF, kind="ExternalInput")
    idb_d = nc.dram_tensor("id_bf", [128, 128], BF, kind="ExternalInput")
    idf_d = nc.dram_tensor("id_f32", [128, 128], F32, kind="ExternalInput")
    out_d = nc.dram_tensor("out", [TSL, E], BF, kind="ExternalOutput")

    def bcast_ap(d, n):
        a = d.ap()
        return bass.AP(tensor=a.tensor, offset=a.offset, ap=[[0, 128], [1, n]])

    with tile.TileContext(nc) as tc:
        with tc.tile_pool(name="dram", bufs=1, space="DRAM") as dram, \
             tc.tile_pool(name="consts", bufs=1) as consts:

            a2a_in = [dram.tile([n_cores, 128, HSL], BF, name=f"a2a_in{b}")
                      for b in range(B)]
            a2a_out = [dram.tile([n_cores, 128, HSL], BF, name=f"a2a_out{b}")
                       for b in range(B)]

            # ---- attention-critical DMAs first on the sync queue --------
            wq_sb = consts.tile([128, EO, HPC * HS], F8)
            nc.sync.dma_start(wq_sb[:], wq_d.ap())
            wk_sb = consts.tile([128, EO, HPC * HS], F8)
            wv_sb = consts.tile([128, EO, HPC * HS], F8)
            masks_sb = consts.tile([128, 128], BF)
            nc.scalar.dma_start(masks_sb[:], masks_d.ap())
            idb_sb = consts.tile([128, 128], BF)
            nc.scalar.dma_start(idb_sb[:], idb_d.ap())
            idf_sb = consts.tile([128, 128], F32)
            nc.gpsimd.dma_start(idf_sb[:], idf_d.ap())
            eps1_sb = consts.tile([128, 1], F32)
            nc.vector.memset(eps1_sb[:], S_LN1_IN * S_LN1_IN * EPS)
            eps2_sb = consts.tile([128, 1], F32)
            nc.vector.memset(eps2_sb[:], S_FF * S_FF * EPS)
            ones_row = consts.tile([1, 128], BF)
            nc.vector.memset(ones_row[:], 1.0)
            # small broadcast vectors on the gpsimd queue (tiny reads)
            b1_sb = consts.tile([128, FO], F32)
            nc.gpsimd.dma_start(b1_sb[:], b1s_d.ap())
            bo_bc = consts.tile([128, E], BF)
            nc.gpsimd.dma_start(bo_bc[:], bcast_ap(bo_d, E))
            b2r_sb = consts.tile([1, E], BF)
            nc.gpsimd.dma_start(b2r_sb[:], b2r_d.ap())
            g1_bc = consts.tile([128, E], BF)
            nc.gpsimd.dma_start(g1_bc[:], bcast_ap(g1_d, E))
            be1_bc = consts.tile([128, E], BF)
            nc.gpsimd.dma_start(be1_bc[:], bcast_ap(be1_d, E))
            g2_bc = consts.tile([128, E], BF)
            nc.gpsimd.dma_start(g2_bc[:], bcast_ap(g2_d, E))
            be2_bc = consts.tile([128, E], BF)
            nc.gpsimd.dma_start(be2_bc[:], bcast_ap(be2_d, E))

            # back-half weights; DMAs issued after qkv0 via WAR memset gate
            # (w1 is streamed inside ffn1 -- read exactly once)
            wo_sb = consts.tile([128, EO, E], F8)
            w2_sb = consts.tile([128, FO, E], BF)
            xpb_sb = consts.tile([128, TSL // 128, E], F32)  # 256*(x+bo)

            # x1f lives across attention (wo0 is interleaved into attn1)
            bh1_cm = tc.tile_pool(name="bh1", bufs=1)
            bh1 = bh1_cm.__enter__()
            x1f = bh1.tile([128, TSL // 128, E], F32, tag="x1f")
            hcT8 = [bh1.tile([128, EO, HSL], F8, tag=f"hcT8_{h2}",
                             name=f"hcT8_{h2}") for h2 in range(B)]

            with tc.tile_pool(name="att_big", bufs=1) as att_big, \
                 tc.tile_pool(name="att_qkv", bufs=2) as att_qkv, \
                 tc.tile_pool(name="att_pt", bufs=3) as att_pt, \
                 tc.tile_pool(name="att_small", bufs=4) as att_small, \
                 tc.tile_pool(name="ps_big", bufs=2, space="PSUM") as ps_big, \
                 tc.tile_pool(name="ps_small", bufs=2, space="PSUM") as ps_small, \
                 tc.tile_pool(name="ps_av", bufs=2, space="PSUM") as ps_av:
                ps_qk = ps_s = ps_big          # 2x 2-bank slots (tag "qs")
                ps_v = ps_tp = ps_small        # shared 1-bank slots

                xT_sb = att_big.tile([128, NTOK // 512, EO, 512], F8, tag="xT")
                nc.sync.dma_start(xT_sb[:, 0], xT_d.ap()[:, 0])
                nc.sync.dma_start(wk_sb[:], wk_d.ap())
                nc.sync.dma_start(wv_sb[:], wv_d.ap())
                for sl_i in range(1, NTOK // 512):
                    nc.sync.dma_start(xT_sb[:, sl_i], xT_d.ap()[:, sl_i])

                qkv_tiles = {}

                def alloc_qkv(b):
                    qkv_tiles[b] = (
                        att_qkv.tile([128, T], BF, tag="qT", name=f"qT{b}"),
                        att_qkv.tile([128, T], BF, tag="kT", name=f"kT{b}"),
                        att_qkv.tile([128, T // 128, 2 * (HS + 1)], BF,
                                     tag="v", name=f"v{b}"),
                    )

                def emit_qk_chunk(b, ci):
                    qT_sb, kT_sb, _ = qkv_tiles[b]
                    cg = b * TC + ci
                    qk_ps = ps_qk.tile([128, 2, 512], F32, tag="qs",
                                       name=f"qk{b}_{ci}")
                    for ep in range(EO // 2):
                        nc.tensor.matmul(
                            qk_ps[:, 0, :], wq_sb[:, 2 * ep:2 * ep + 2, :],
                            xT_sb[:, cg, 2 * ep:2 * ep + 2, :],
                            start=ep == 0, stop=ep == EO // 2 - 1,
                            perf_mode=DR)
                    for ep in range(EO // 2):
                        nc.tensor.matmul(
                            qk_ps[:, 1, :], wk_sb[:, 2 * ep:2 * ep + 2, :],
                            xT_sb[:, cg, 2 * ep:2 * ep + 2, :],
                            start=ep == 0, stop=ep == EO // 2 - 1,
                            perf_mode=DR)
                    nc.vector.tensor_copy(
                        qT_sb[:, 512 * ci:512 * ci + 512], qk_ps[:, 0, :])
                    nc.vector.tensor_copy(
                        kT_sb[:, 512 * ci:512 * ci + 512], qk_ps[:, 1, :])

                def emit_v_unit(b, ci, k2):
                    _, _, v_sb = qkv_tiles[b]
                    cg = b * TC + ci
                    vp = ps_v.tile([128, 512], F32, tag="vtp",
                                   name=f"vp{b}_{ci}_{k2}")
                    vps = vp[:, 0:128]
                    for ep in range(EO // 2):
                        nc.tensor.matmul(
                            vps, xT_sb[:, cg, 2 * ep:2 * ep + 2,
                                       128 * k2:128 * (k2 + 1)],
                            wv_sb[:, 2 * ep:2 * ep + 2, :],
                            start=ep == 0, stop=ep == EO // 2 - 1,
                            perf_mode=DR)
                    ts_ = 4 * ci + k2
                    vrow = v_sb[:, ts_, :]
                    ones_view = bass.AP(
                        tensor=vrow.tensor, offset=vrow.offset + HS,
                        ap=[vrow.ap[0], [HS + 1, 2]])
                    nc.vector.memset(ones_view, 1.0)
                    dst = bass.AP(
                        tensor=vrow.tensor, offset=vrow.offset,
                        ap=[vrow.ap[0], [HS + 1, 2], [1, HS]])
                    nc.vector.tensor_copy(
                        dst, vps.rearrange("p (h d) -> p h d", h=2))

                def emit_attn_chunk(b, i, fillers):
                    qT_sb, kT_sb, v_sb = qkv_tiles[b]
                    av_ps = [ps_av.tile([128, 2, 2, HS + 1], F32, tag="av",
                                        name=f"av{b}_{i}_{p}")
                             for p in range(2)]
                    nj = 4 * i + 4
                    nf = len(fillers)
                    fdone = 0
                    s_tiles = {}

                    def emit_scores(j):
                        s_ps = ps_s.tile([128, 2, 512], F32, tag="qs",
                                         name=f"s{b}_{i}_{j}")
                        for h in range(2):
                            nc.tensor.matmul(
                                s_ps[:, h, :],
                                kT_sb[64 * h:64 * h + 64,
                                      128 * j:128 * j + 128],
                                qT_sb[64 * h:64 * h + 64,
                                      512 * i:512 * i + 512],
                                start=True, stop=True)
                        s_tiles[j] = s_ps

                    emit_scores(0)
                    for j in range(nj):
                        # keep the score stream one j ahead so the scalar
                        # engine's exp pipeline never starves
                        if j + 1 < nj:
                            emit_scores(j + 1)
                        q = j - 4 * i
                        t0_ = 128 * q if q > 0 else 0
                        s_ps = s_tiles.pop(j)
                        pt = att_pt.tile([128, 2, 512], BF, tag="pt",
                                         name=f"pt{b}_{i}_{j}")
                        nc.scalar.activation(
                            pt[:, :, t0_:], s_ps[:, :, t0_:], AF.Exp,
                            scale=1.0 / (np.sqrt(HS) * SW * SW))
                        if q >= 0:
                            nc.vector.tensor_tensor(
                                pt[:, :, 128 * q:128 * (q + 1)],
                                pt[:, :, 128 * q:128 * (q + 1)],
                                masks_sb[:, None, :].to_broadcast((128, 2, 128)),
                                OP.mult)
                        for k2 in range(4):
                            if j > 4 * i + k2:
                                continue
                            for h in range(2):
                                nc.tensor.matmul(
                                    av_ps[k2 // 2][:, k2 % 2, h, :],
                                    pt[:, h, 128 * k2:128 * (k2 + 1)],
                                    v_sb[:, j, (HS + 1) * h:(HS + 1) * (h + 1)],
                                    start=(j == 0 and h == 0 and k2 % 2 == 0),
                                    stop=j == 4 * i + k2)
                        # sprinkle independent PE work between j iterations
                        want = nf * (j + 1) // nj
                        while fdone < want:
                            fillers[fdone]()
                            fdone += 1
                    for k2 in range(4):
                        avp = av_ps[k2 // 2][:, k2 % 2, :, :]
                        recip = att_small.tile([128, 2], F32, tag="recip")
                        nc.vector.reciprocal(recip[:], avp[:, :, HS])
                        onorm = att_small.tile([128, 128], BF, tag="onorm")
                        for h in range(2):
                            nc.vector.tensor_scalar_mul(
                                onorm[:, 64 * h:64 * h + 64],
                                avp[:, h, 0:HS], recip[:, h:h + 1])
                        tp = ps_tp.tile([128, 512], F32, tag="vtp",
                                        name=f"tp{b}_{i}_{k2}")
                        tpb = tp[:, 0:128].bitcast(BF)[:, 0:128]
                        nc.tensor.transpose(tpb, onorm[:], idb_sb[:])
                        ot = att_small.tile([128, 128], BF, tag="ot")
                        nc.vector.tensor_copy(ot[:], tpb)
                        g2_ = 512 * i + 128 * k2
                        nc.scalar.dma_start(
                            a2a_in[b][g2_ // HSL, :,
                                      (g2_ % HSL):(g2_ % HSL) + 128],
                            ot[:])

                # ---- batch 0 qkv ----
                with nc.named_scope("qkv0"):
                    alloc_qkv(0)
                    for ci in range(TC):
                        emit_qk_chunk(0, ci)
                        for k2 in range(4):
                            emit_v_unit(0, ci, k2)

                # gate the big prefetches behind qkv0 (vector engine reaches
                # these memsets ~25us in; keeps the startup DMA burst small)
                nc.vector.memset(wo_sb[:, 0, 0:1], 0.0)
                nc.vector.memset(xpb_sb[:, 0, 0:1], 0.0)
                nc.sync.dma_start(wo_sb[:], wo_d.ap())
                nc.sync.dma_start(xpb_sb[:], xsl_d.ap())

                # ---- attn0 with qkv1 interleaved as j-level fillers ----
                alloc_qkv(1)
                emit_qk_chunk(1, 0)
                with nc.named_scope("attn0"):
                    for i in range(TC):
                        fillers = []
                        if i + 1 < TC:
                            fillers.append(
                                lambda ci=i + 1: emit_qk_chunk(1, ci))
                        fillers += [
                            (lambda ci=i, k2=k2: emit_v_unit(1, ci, k2))
                            for k2 in range(4)]
                        emit_attn_chunk(0, i, fillers)
                        if i == 0:
                            # release the big w2 prefetch mid-attention
                            nc.vector.memset(w2_sb[:, 0, 0:1], 0.0)
                            nc.sync.dma_start(w2_sb[:], w2_d.ap())

                with nc.named_scope("a2a0"):
                    nc.gpsimd.collective_compute(
                        "AllToAll", mybir.AluOpType.bypass,
                        replica_groups=[list(range(n_cores))],
                        ins=[a2a_in[0].opt()], outs=[a2a_out[0].opt()])
                hcT0 = att_big.tile([128, EO, HSL], BF, tag="hcT")
                nc.sync.dma_start(
                    hcT0[:], a2a_out[0][:].rearrange("i p t -> p i t"))
                nc.gpsimd.tensor_copy(hcT8[0][:], hcT0[:])
                # ---- attn1 (xpb DVE prep spread at chunk boundaries) --
                with nc.named_scope("attn1"):
                    for i in range(TC):
                        emit_attn_chunk(1, i, [])
                        m_ = i
                        nc.vector.tensor_scalar_mul(xpb_sb[:, m_, :],
                                                    xpb_sb[:, m_, :], S_LN1_IN)
                        nc.vector.tensor_tensor(xpb_sb[:, m_, :],
                                                xpb_sb[:, m_, :],
                                                bo_bc[:], OP.add)

                with nc.named_scope("a2a1"):
                    nc.gpsimd.collective_compute(
                        "AllToAll", mybir.AluOpType.bypass,
                        replica_groups=[list(range(n_cores))],
                        ins=[a2a_in[1].opt()], outs=[a2a_out[1].opt()])
                hcT1 = att_big.tile([128, EO, HSL], BF, tag="hcT")
                nc.sync.dma_start(
                    hcT1[:], a2a_out[1][:].rearrange("i p t -> p i t"))
                nc.gpsimd.tensor_copy(hcT8[1][:], hcT1[:])

            # ================= back half =================
            with tc.tile_pool(name="bh2", bufs=1) as bh2, \
                 tc.tile_pool(name="bh_small", bufs=4) as bh_small:

                x1T = bh2.tile([128, EO, TSL], F8, tag="x1T")
                hT = bh2.tile([128, FO, TSL], BF, tag="hT")
                out_bf = bh2.tile([128, TSL // 128, E], BF, tag="out_bf")
                out_dst = out_d.ap().rearrange("(m p) e -> p m e", p=128)

                def layernorm(buf_m, g_bc, be_bc, eps_sb, out_m=None):
                    stats = bh_small.tile([128, 2, 6], F32, tag="stats")
                    for s2 in range(2):
                        nc.vector.bn_stats(stats[:, s2, :],
                                           buf_m[:, 512 * s2:512 * (s2 + 1)])
                    mv = bh_small.tile([128, 2], F32, tag="mv")
                    nc.vector.bn_aggr(mv[:], stats[:])
                    std = bh_small.tile([128, 1], F32, tag="std")
                    nc.scalar.activation(std[:], mv[:, 1:2], AF.Sqrt,
                                         bias=eps_sb[:, 0:1])
                    rstd = bh_small.tile([128, 1], F32, tag="rstd")
                    nc.vector.reciprocal(rstd[:], std[:])
                    nc.vector.tensor_scalar(
                        buf_m[:], buf_m[:], mv[:, 0:1], rstd[:],
                        op0=OP.subtract, op1=OP.mult)
                    nc.vector.tensor_tensor(buf_m[:], buf_m[:], g_bc[:], OP.mult)
                    nc.vector.tensor_tensor(out_m if out_m is not None
                                            else buf_m[:],
                                            buf_m[:], be_bc[:], OP.add)

                with tc.tile_pool(name="ps_wo1", bufs=2, space="PSUM") as ps_wo1, \
                     tc.tile_pool(name="ps_tp2", bufs=2, space="PSUM") as ps_tp2:
                    with nc.named_scope("ln1_0"):
                        for m2 in range(M2N):
                            for n in range(2):
                                wo_ps = ps_wo1.tile([128, 512], F32, tag="wo")
                                for hp in range(EO // 2):
                                    nc.tensor.matmul(
                                        wo_ps[:],
                                        hcT8[0][:, 2 * hp:2 * hp + 2,
                                                128 * m2:128 * (m2 + 1)],
                                        wo_sb[:, 2 * hp:2 * hp + 2,
                                              512 * n:512 * (n + 1)],
                                        start=hp == 0, stop=hp == EO // 2 - 1,
                                        perf_mode=DR)
                                sl = slice(512 * n, 512 * (n + 1))
                                nc.vector.tensor_tensor(
                                    x1f[:, m2, sl], wo_ps[:], xpb_sb[:, m2, sl],
                                    OP.add)
                        for m2 in range(M2N):
                            layernorm(x1f[:, m2, :], g1_bc, be1_bc, eps1_sb)
                        for m2 in range(M2N):
                            for eo in range(EO):
                                tp2 = ps_tp2.tile([128, 128], F32, tag="tp2")
                                nc.tensor.transpose(
                                    tp2[:], x1f[:, m2, 128 * eo:128 * (eo + 1)],
                                    idf_sb[:])
                                nc.vector.tensor_copy(
                                    x1T[:, eo, 128 * m2:128 * (m2 + 1)], tp2[:])

                    with nc.named_scope("wo_ln1_1"):
                        for m2 in range(M2N):
                            for n in range(2):
                                m = M2N + m2
                                wo_ps = ps_wo1.tile([128, 512], F32, tag="wo")
                                for hp in range(EO // 2):
                                    nc.tensor.matmul(
                                        wo_ps[:],
                                        hcT8[1][:, 2 * hp:2 * hp + 2,
                                                128 * m2:128 * (m2 + 1)],
                                        wo_sb[:, 2 * hp:2 * hp + 2,
                                              512 * n:512 * (n + 1)],
                                        start=hp == 0, stop=hp == EO // 2 - 1,
                                        perf_mode=DR)
                                sl = slice(512 * n, 512 * (n + 1))
                                nc.vector.tensor_tensor(
                                    x1f[:, m, sl], wo_ps[:], xpb_sb[:, m, sl],
                                    OP.add)
                        for m2 in range(M2N):
                            m = M2N + m2
                            layernorm(x1f[:, m, :], g1_bc, be1_bc, eps1_sb)
                        for m2 in range(M2N):
                            m = M2N + m2
                            for eo in range(EO):
                                tp2 = ps_tp2.tile([128, 128], F32, tag="tp2")
                                nc.tensor.transpose(
                                    tp2[:], x1f[:, m, 128 * eo:128 * (eo + 1)],
                                    idf_sb[:])
                                nc.vector.tensor_copy(
                                    x1T[:, eo, 128 * m:128 * (m + 1)], tp2[:])

                with nc.named_scope("ffn"), \
                     tc.tile_pool(name="bh_w", bufs=6) as bh_w, \
                     tc.tile_pool(name="ps_f1", bufs=3, space="PSUM") as ps_f1, \
                     tc.tile_pool(name="ps_f2", bufs=1, space="PSUM") as ps_f2:

                    def f2_banks(mlist):
                        ps = {}
                        for m in mlist:
                            for n in range(2):
                                ps[m, n] = ps_f2.tile(
                                    [128, 512], F32, tag=f"f2_{m % 2}_{n}",
                                    name=f"f2_{m}_{n}")
                                nc.tensor.matmul(
                                    ps[m, n][:], ones_row[:],
                                    b2r_sb[:, 512 * n:512 * (n + 1)],
                                    start=True, stop=False)
                        return ps

                    def f2_epilogue(ps, mlist):
                        for m in mlist:
                            for n in range(2):
                                sl = slice(512 * n, 512 * (n + 1))
                                nc.vector.tensor_tensor(
                                    x1f[:, m, sl], ps[m, n][:], x1f[:, m, sl],
                                    OP.add)
                            layernorm(x1f[:, m, :], g2_bc, be2_bc, eps2_sb,
                                      out_m=out_bf[:, m, :])
                            nc.sync.dma_start(out_dst[:, m, :],
                                              out_bf[:, m, :])

                    # phase 1: ffn1 (all fo) pipelined with ffn2 for m=0,1
                    ps01 = f2_banks([0, 1])
                    for fo in range(FO):
                        w1t = bh_w.tile([128, EO, 128], F8, tag="w1t")
                        nc.gpsimd.dma_start(w1t[:], w1_d.ap()[:, fo])
                        f1_ps = ps_f1.tile([128, TSL], F32, tag="f1")
                        for ep in range(EO // 2):
                            nc.tensor.matmul(
                                f1_ps[:], w1t[:, 2 * ep:2 * ep + 2, :],
                                x1T[:, 2 * ep:2 * ep + 2, :],
                                start=ep == 0, stop=ep == EO // 2 - 1,
                                perf_mode=DR)
                        nc.scalar.activation(hT[:, fo, :], f1_ps[:], AF.Relu,
                                             bias=b1_sb[:, fo:fo + 1],
                                             scale=1.0 / (S_LN1_OUT * SW))
                        for m in range(2):
                            for n in range(2):
                                nc.tensor.matmul(
                                    ps01[m, n][:],
                                    hT[:, fo, 128 * m:128 * (m + 1)],
                                    w2_sb[:, fo, 512 * n:512 * (n + 1)],
                                    start=False, stop=fo == FO - 1)
                    f2_epilogue(ps01, [0, 1])

                    # phase 2: ffn2 for m=2,3 (hT fully materialized)
                    ps23 = f2_banks([2, 3])
                    for fo in range(FO):
                        for m in range(2, 4):
                            for n in range(2):
                                nc.tensor.matmul(
                                    ps23[m, n][:],
                                    hT[:, fo, 128 * m:128 * (m + 1)],
                                    w2_sb[:, fo, 512 * n:512 * (n + 1)],
                                    start=False, stop=fo == FO - 1)
                    f2_epilogue(ps23, [2, 3])

            bh1_cm.__exit__(None, None, None)

    nc.compile()
    return nc


def _make_in_maps(inputs):
    x = np.asarray(inputs["x"], dtype=np.float32)
    Wq = np.asarray(inputs["Wq"], dtype=np.float32)
    Wk = np.asarray(inputs["Wk"], dtype=np.float32)
    Wv = np.asarray(inputs["Wv"], dtype=np.float32)
    Wo = np.asarray(inputs["Wo"], dtype=np.float32)

    xflat = x.reshape(NTOK, E)
    xT = np.ascontiguousarray(
        xflat.reshape(NTOK // 512, 512, EO_, 128).transpose(3, 0, 2, 1)
    ).astype(FP8)
    wo = np.ascontiguousarray(
        (SW * Wo).reshape(EO_, 128, E).transpose(1, 0, 2)).astype(FP8)
    w1 = np.ascontiguousarray(
        (SW * np.asarray(inputs["W1"], dtype=np.float32))
        .reshape(EO_, 128, FO_, 128).transpose(1, 2, 0, 3)
    ).astype(FP8)
    w2 = np.ascontiguousarray(
        (S_FF * np.asarray(inputs["W2"], dtype=np.float32))
        .reshape(FO_, 128, E).transpose(1, 0, 2)).astype(BF16)
    b1s = np.ascontiguousarray(
        np.asarray(inputs["b1"], dtype=np.float32).reshape(FO_, 128).T)

    srow = np.arange(128)[:, None]
    tcol = np.arange(128)[None, :]
    masks = np.ascontiguousarray((srow <= tcol).astype(np.float32)).astype(BF16)

    ident = np.eye(128, dtype=np.float32)

    common = {
        "xT": xT,
        "wo": wo,
        "w1": w1,
        "w2": w2,
        "b1s": b1s,
        "bo": (S_LN1_IN * np.asarray(inputs["bo"], dtype=np.float32)
               ).astype(BF16),
        "b2r": (S_FF * np.asarray(inputs["b2"], dtype=np.float32)
        ).astype(BF16).reshape(1, E),
        "g1": (S_LN1_OUT * np.asarray(inputs["g1"], dtype=np.float32)
               ).astype(BF16),
        "be1": (S_LN1_OUT * np.asarray(inputs["be1"], dtype=np.float32)
                ).astype(BF16),
        "g2": np.asarray(inputs["g2"], dtype=np.float32).astype(BF16),
        "be2": np.asarray(inputs["be2"], dtype=np.float32).astype(BF16),
        "masks": masks,
        "id_bf": ident.astype(BF16),
        "id_f32": ident,
    }
    in_maps = []
    for c in range(N_CORES):
        m = dict(common)
        def tile_w(W):
            wc = np.concatenate([W[2 * c], W[2 * c + 1]], axis=1)
            return np.ascontiguousarray(
                (SW * wc).reshape(EO_, 128, 128).transpose(1, 0, 2)).astype(FP8)
        m["wq"] = tile_w(Wq)
        m["wk"] = tile_w(Wk)
        m["wv"] = tile_w(Wv)
        rows = np.concatenate([
            xflat[HSL * c:HSL * (c + 1)],
            xflat[T + HSL * c:T + HSL * (c + 1)]], axis=0)
        m["x_slice"] = np.ascontiguousarray(
            rows.reshape(TSL // 128, 128, E).transpose(1, 0, 2))
        in_maps.append(m)
    return in_maps


def _enable_trace_hook():
    """Register the axon NTFF profile hook (synthesize antenv.axon_hooks)."""
    import types
    import antenv  # noqa: F401

    if "antenv.axon_hooks" not in sys.modules:
        mod = types.ModuleType("antenv.axon_hooks")
        mod._hook = None
        mod.set_axon_ntff_profile_hook = lambda h: setattr(mod, "_hook", h)
        mod.get_axon_ntff_profile_hook = lambda: mod._hook
        sys.modules["antenv.axon_hooks"] = mod
        antenv.axon_hooks = mod
    mod = sys.modules["antenv.axon_hooks"]
    if mod.get_axon_ntff_profile_hook() is None:
        if "/root/.axon_site" not in sys.path:
            sys.path.insert(0, "/root/.axon_site")
        from trn_agent_boot.trn_boot import _ntff_profile_via_ctypes
        mod.set_axon_ntff_profile_hook(
            _ntff_profile_via_ctypes("/opt/axon/libaxon_pjrt.so"))


def run(inputs, trace=False):
    """Returns (full_output [B,T,E] f32, BassKernelResults)."""
    from concourse import bass_utils

    if "nc" not in _cache:
        _cache["nc"] = _build()
    nc = _cache["nc"]
    in_maps = _make_in_maps(inputs)
    if trace:
        _enable_trace_hook()
    res = bass_utils.run_bass_kernel_spmd(
        nc, in_maps, core_ids=list(range(N_CORES)), trace=trace)
    full = np.empty((NTOK, E), dtype=np.float32)
    for c in range(N_CORES):
        o = res.results[c]["out"]
        full[HSL * c:HSL * (c + 1)] = o[:HSL]
        full[T + HSL * c:T + HSL * (c + 1)] = o[HSL:]
    return full.reshape(B, T, E), res


def kernel(**inputs):
    out, _ = run(inputs, trace=False)
    return out
